# revision 2
# baseline (speedup 1.0000x reference)
"""Self-attention layer (softmax(X @ X^T) @ X) on 8 Trainium2 NeuronCores.

Data-parallel over batch: each of the 8 cores computes one batch element's
full attention for X of shape [2048, 512].

Per-core algorithm (all matmuls in bf16, accumulation in f32):
  1. Load X, convert to bf16, PE-transpose to get Xt = X^T  [512, 2048].
  2. For each row-block i (128 rows):
       scores S_i = X_i @ X^T        (4 psum chunks of [128, 512])
       m = rowmax(S_i)               (vector engine)
       E_i = exp(S_i - m), l = rowsum (fused on scalar engine, accum_out)
       PE-transpose E_i tiles into E^T  (for the PV matmul's stationary side)
       O_i = (E^T[:, i]^T @ X) / l   (16 accumulating matmuls + scale)
  3. DMA O_i back out.

The loop is software-pipelined: row-block i's QK matmuls are emitted before
row-block i-1's transpose+PV so the tensor engine never waits on softmax.
"""

import os
import numpy as np

B, S, D = 8, 2048, 512
P = 128
NI = S // P  # 16 row blocks
NK = D // P  # 4 contraction tiles for QK
JC = 512     # score column chunk (one psum bank)
NJ = S // JC  # 4 chunks per row block
NSUB = JC // P  # 4 [128,128] subtiles per chunk

_CACHE = {}


def _build_nc():
    from contextlib import ExitStack

    import concourse.bacc as bacc
    import concourse.mybir as mybir
    import concourse.tile as tile
    from concourse import masks

    f32 = mybir.dt.float32
    bf16 = mybir.dt.bfloat16
    AF = mybir.ActivationFunctionType
    ALU = mybir.AluOpType
    AX = mybir.AxisListType

    nc = bacc.Bacc("TRN2", target_bir_lowering=False, debug=False, num_devices=B)
    inp = nc.dram_tensor("inputs", [S, D], f32, kind="ExternalInput").ap()
    out = nc.dram_tensor("out", [S, D], f32, kind="ExternalOutput").ap()

    with tile.TileContext(nc) as tc, ExitStack() as ctx:
        const_pool = ctx.enter_context(tc.tile_pool(name="const", bufs=1))
        persist = ctx.enter_context(tc.tile_pool(name="persist", bufs=1))
        xin_pool = ctx.enter_context(tc.tile_pool(name="xin", bufs=3))
        enat_pool = ctx.enter_context(tc.tile_pool(name="enat", bufs=2 * NJ))
        stat_pool = ctx.enter_context(tc.tile_pool(name="stat", bufs=3))
        osb_pool = ctx.enter_context(tc.tile_pool(name="osb", bufs=3))
        qk_psum = ctx.enter_context(tc.tile_pool(name="qk_psum", bufs=4, space="PSUM"))
        tr_psum = ctx.enter_context(tc.tile_pool(name="tr_psum", bufs=2, space="PSUM"))
        pv_psum = ctx.enter_context(tc.tile_pool(name="pv_psum", bufs=2, space="PSUM"))

        ident = const_pool.tile([P, P], bf16, tag="ident", name="ident")
        masks.make_identity(nc, ident[:])

        X_bf = [
            persist.tile([P, D], bf16, tag=f"xbf{i}", name=f"xbf{i}")
            for i in range(NI)
        ]
        Xt = [
            persist.tile([P, S], bf16, tag=f"xt{k}", name=f"xt{k}") for k in range(NK)
        ]
        E_T = [
            persist.tile([P, S], bf16, tag=f"et{j}", name=f"et{j}") for j in range(NI)
        ]

        # ---- load + convert + transpose X ----
        for i in range(NI):
            xf = xin_pool.tile([P, D], f32, tag="xf", name=f"xf{i}")
            nc.sync.dma_start(xf[:], inp[i * P : (i + 1) * P, :])
            nc.scalar.copy(X_bf[i][:], xf[:])
        for i in range(NI):
            for k in range(NK):
                pt = tr_psum.tile([P, P], bf16, tag="pt", name=f"ptx{i}_{k}")
                nc.tensor.transpose(pt[:], X_bf[i][:, k * P : (k + 1) * P], ident[:])
                nc.vector.tensor_copy(Xt[k][:, i * P : (i + 1) * P], pt[:])

        # ---- pipelined main loop ----
        state = {}

        def emit_front(i):
            # scores for row block i, row max, exp, row sums
            icol = slice(i * P, (i + 1) * P)
            mm = stat_pool.tile([P, NJ], f32, tag=f"mm{i}", name=f"mm{i}")
            psums = []
            for jc in range(NJ):
                ps = qk_psum.tile([P, JC], f32, tag="qk", name=f"qk{i}_{jc}")
                for k in range(NK):
                    nc.tensor.matmul(
                        ps[:],
                        lhsT=Xt[k][:, icol],
                        rhs=Xt[k][:, jc * JC : (jc + 1) * JC],
                        start=(k == 0),
                        stop=(k == NK - 1),
                    )
                nc.vector.tensor_reduce(mm[:, jc : jc + 1], ps[:], axis=AX.X, op=ALU.max)
                psums.append(ps)
            negm = stat_pool.tile([P, 1], f32, tag=f"negm{i}", name=f"negm{i}")
            nc.vector.tensor_reduce(negm[:], mm[:], axis=AX.X, op=ALU.max, negate=True)
            lpart = stat_pool.tile([P, NJ], f32, tag=f"lpart{i}", name=f"lpart{i}")
            enats = []
            for jc in range(NJ):
                en = enat_pool.tile([P, JC], bf16, tag="enat", name=f"enat{i}_{jc}")
                nc.scalar.activation(
                    en[:],
                    psums[jc][:],
                    AF.Exp,
                    bias=negm[:],
                    scale=1.0,
                    accum_out=lpart[:, jc : jc + 1],
                )
                enats.append(en)
            l = stat_pool.tile([P, 1], f32, tag=f"l{i}", name=f"l{i}")
            nc.vector.tensor_reduce(l[:], lpart[:], axis=AX.X, op=ALU.add)
            linv = stat_pool.tile([P, 1], f32, tag=f"linv{i}", name=f"linv{i}")
            nc.vector.reciprocal(linv[:], l[:])
            state[i] = (enats, linv)

        def emit_back(i):
            # transpose E row-block i into E_T, then the PV matmul for block i
            icol = slice(i * P, (i + 1) * P)
            enats, linv = state.pop(i)
            for jc in range(NJ):
                for kk in range(NSUB):
                    j = jc * NSUB + kk
                    pt = tr_psum.tile([P, P], bf16, tag="pt", name=f"pte{i}_{j}")
                    nc.tensor.transpose(
                        pt[:], enats[jc][:, kk * P : (kk + 1) * P], ident[:]
                    )
                    nc.vector.tensor_copy(E_T[j][:, icol], pt[:])
            po = pv_psum.tile([P, D], f32, tag="pv", name=f"pv{i}")
            for j in range(NI):
                nc.tensor.matmul(
                    po[:],
                    lhsT=E_T[j][:, icol],
                    rhs=X_bf[j][:],
                    start=(j == 0),
                    stop=(j == NI - 1),
                )
            osb = osb_pool.tile([P, D], f32, tag="osb", name=f"osb{i}")
            nc.scalar.mul(osb[:], po[:], linv[:])
            nc.sync.dma_start(out[i * P : (i + 1) * P, :], osb[:])

        for i in range(NI + 1):
            if i < NI:
                emit_front(i)
            if i >= 1:
                emit_back(i - 1)

    nc.compile()
    return nc


def _maybe_install_trace_hook():
    """Install the NTFF profile hook (test/profiling only; optional)."""
    import sys
    import types

    try:
        from antenv.axon_hooks import get_axon_ntff_profile_hook  # noqa: F401

        return  # already available
    except ImportError:
        pass
    try:
        mod = types.ModuleType("antenv.axon_hooks")
        _hook = [None]
        mod.set_axon_ntff_profile_hook = lambda h: _hook.__setitem__(0, h)
        mod.get_axon_ntff_profile_hook = lambda: _hook[0]
        sys.modules["antenv.axon_hooks"] = mod
        import antenv

        antenv.axon_hooks = mod
        from trn_agent_boot.trn_boot import _ntff_profile_via_ctypes

        mod.set_axon_ntff_profile_hook(
            _ntff_profile_via_ctypes("/opt/axon/libaxon_pjrt.so")
        )
    except Exception:
        pass


def kernel(inputs: np.ndarray) -> np.ndarray:
    from concourse.bass_utils import run_bass_kernel_spmd

    x = np.ascontiguousarray(np.asarray(inputs, dtype=np.float32))
    assert x.shape == (B, S, D), f"unexpected input shape {x.shape}"

    if "nc" not in _CACHE:
        _CACHE["nc"] = _build_nc()
    nc = _CACHE["nc"]

    trace = bool(int(os.environ.get("ATT_KERNEL_TRACE", "0")))
    if trace:
        _maybe_install_trace_hook()

    in_maps = [{"inputs": x[b]} for b in range(B)]
    res = run_bass_kernel_spmd(nc, in_maps, core_ids=list(range(B)), trace=trace)
    kernel.last_exec_time_ns = res.exec_time_ns
    return np.stack([res.results[b]["out"] for b in range(B)], axis=0)


kernel.last_exec_time_ns = None
